# revision 2
# baseline (speedup 1.0000x reference)
"""TRN2 Bass kernel v6 for nn_CustomAttnProcessor (B=8, S=1024, C=1280, H=20).

Strategy (v1 baseline 442 us -> v4 372 us -> this):
  - Batch-parallel: one batch element per NeuronCore, no collectives.
  - All SBUF operands bf16 (rel err ~6e-3 vs 2e-2 budget).
  - Chunked input DMAs + PE warm-up matmuls during the DMA fill.
  - Q/K projection matmuls woven INTO the attention stage loop so exp
    (ScalarE, ~184 us total) hides behind projection matmuls.
  - Attention processes HEAD PAIRS: the two heads of a c-tile use disjoint
    PE row halves (partitions 0-63 / 64-127), so their K=64 scores matmuls
    run CONCURRENTLY via tile_position row tiling.  Both heads' scores for
    one q-half live in one [128,1024] PSUM tile -> one exp per (kt, qh)
    keeps ScalarE at its N=1024 efficiency.
  - PV per head accumulates [128, 512] per q-half (stationary = [64 v cols |
    64 ones cols], M=128): rows 64..127 of the output are the softmax
    denominator pre-broadcast; normalize = copy + recip + mul, all >=64-lane.
  - PSUM: scores "ps" [128,1024] x2 (4 banks) + PV "pvo" [128,512] x3
    (3 banks) + woven-projection "held" [128,512] x1 (1 bank) = 8 banks.
"""

import os
import sys
import types

import numpy as np

B, S, C, H = 8, 1024, 1280, 20
D = C // H              # 64 head dim
P = 128
NCT = C // P            # 10 c-tiles
NST = S // P            # 8 s-tiles
SCALE = float(D) ** -0.5
DEPTH = 2               # PV stage lag behind scores/exp

_cache = {}

last_exec_time_ns = None


def _install_profile_hook():
    """antenv.axon_hooks is absent in this container; reconstruct it from
    trn_agent_boot so BASS_TRACE=1 profiling works."""
    if "antenv.axon_hooks" in sys.modules:
        return
    mod = types.ModuleType("antenv.axon_hooks")
    state = {"hook": None}
    mod.set_axon_ntff_profile_hook = lambda h: state.update(hook=h)
    mod.get_axon_ntff_profile_hook = lambda: state["hook"]
    sys.modules["antenv.axon_hooks"] = mod
    try:
        from trn_agent_boot.trn_boot import _ntff_profile_via_ctypes

        hook = _ntff_profile_via_ctypes("/opt/axon/libaxon_pjrt.so")
        if hook is not None:
            mod.set_axon_ntff_profile_hook(hook)
    except Exception:
        pass


def _build():
    import concourse.mybir as mybir
    from concourse import bacc
    from concourse.tile import TileContext

    f32 = mybir.dt.float32
    bf16 = mybir.dt.bfloat16
    EXP = mybir.ActivationFunctionType.Exp
    IDN = mybir.ActivationFunctionType.Identity

    nc = bacc.Bacc()
    xt = nc.declare_dram_parameter("xt", [NCT, P, S], bf16, isOutput=False)
    wvt = nc.declare_dram_parameter("wvt", [NCT, P, C], bf16, isOutput=False)
    wqt = nc.declare_dram_parameter("wqt", [NCT, P, NCT, P], bf16, isOutput=False)
    wkt = nc.declare_dram_parameter("wkt", [NCT, P, NCT, P], bf16, isOutput=False)
    wot = nc.declare_dram_parameter("wot", [NCT, P, NCT, P], bf16, isOutput=False)
    bo = nc.declare_dram_parameter("bo", [P, NCT], f32, isOutput=False)
    yt = nc.declare_dram_parameter("yt", [NCT, P, S], f32, isOutput=True)

    with TileContext(nc) as tc:
        with (
            tc.tile_pool(name="cst", bufs=1) as c_pool,
            tc.tile_pool(name="xa", bufs=1) as x_pool,
            tc.tile_pool(name="qk", bufs=1) as qk_pool,
            tc.tile_pool(name="vv", bufs=1) as v_pool,
            tc.tile_pool(name="att", bufs=1) as a_pool,
            tc.tile_pool(name="pt", bufs=6) as pt_pool,
            tc.tile_pool(name="wcp", bufs=3) as w_pool,
            tc.tile_pool(name="pos", bufs=4) as po_pool,
            tc.tile_pool(name="nrm", bufs=4) as n_pool,
            tc.tile_pool(name="yts", bufs=3) as y_pool,
            tc.tile_pool(name="psm", bufs=1, space="PSUM") as psum,
        ):
            # ---- PE warm-up: junk matmuls on a memset tile (no DMA dep) ----
            junk = c_pool.tile([P, 512], bf16, tag="junk")
            nc.vector.memset(junk, 0.5)

            # ---- constants ----
            bo_sb = c_pool.tile([P, NCT], f32)
            nc.sync.dma_start(out=bo_sb, in_=bo[:, :])
            # ---- chunked input DMAs (xt/wvt interleaved for early start) ----
            xt_sb = []
            wvt_sb = []
            for k in range(NCT):
                xk = x_pool.tile([P, S], bf16, tag=f"xt{k}", name=f"xts{k}")
                nc.sync.dma_start(out=xk, in_=xt[k])
                xt_sb.append(xk)
                wk_ = x_pool.tile([P, C], bf16, tag=f"wv{k}", name=f"wvs{k}")
                nc.sync.dma_start(out=wk_, in_=wvt[k])
                wvt_sb.append(wk_)

            # V tiles: per-head 128-wide slot = [64 v cols | 64 ones cols].
            # memset(1.0) provides the ones half; evictions overwrite v halves.
            # M=128 stationary: PV emits the softmax denominator pre-broadcast
            # to rows 64..127.
            v_sb = []
            for m in range(NST):
                vm = v_pool.tile([P, H * P], bf16, tag=f"v{m}", name=f"vsb{m}")
                nc.vector.memset(vm, 1.0)
                v_sb.append(vm)

            warm = psum.tile([P, 512], f32, tag="held", bufs=1, name="warm")
            for w_i in range(30):
                nc.tensor.matmul(
                    warm, lhsT=junk[:, 0:P], rhs=junk, start=True, stop=True,
                )

            qt_sb = [
                qk_pool.tile([P, S], bf16, tag=f"q{i}", name=f"qts{i}")
                for i in range(NCT)
            ]
            kt_sb = [
                qk_pool.tile([P, S], bf16, tag=f"k{i}", name=f"kts{i}")
                for i in range(NCT)
            ]
            at_sb = [
                a_pool.tile([P, S], bf16, tag=f"a{i}", name=f"ats{i}")
                for i in range(NCT)
            ]

            # ---- V projection ----
            # main: o 0..1023 (heads 0-15) in a [128,1024] ps tile per m-unit;
            # tail: o 1024..1279 (heads 16-19), 2 m-units share one [128,512]
            # pvo tile.
            for g in range(4):
                tail = psum.tile([P, 512], f32, tag="pvo", bufs=3, name=f"vtail{g}")
                for mm_i in range(2):
                    m = g * 2 + mm_i
                    main = psum.tile([P, S], f32, tag="ps", bufs=2, name=f"vmain{m}")
                    for k in range(NCT):
                        lhs = xt_sb[k][:, m * P:(m + 1) * P]
                        st, sp = (k == 0), (k == NCT - 1)
                        nc.tensor.matmul(
                            main[:, 0:512], lhsT=lhs, rhs=wvt_sb[k][:, 0:512],
                            start=st, stop=sp,
                        )
                        nc.tensor.matmul(
                            main[:, 512:1024], lhsT=lhs, rhs=wvt_sb[k][:, 512:1024],
                            start=st, stop=sp,
                        )
                        nc.tensor.matmul(
                            tail[:, mm_i * 256:(mm_i + 1) * 256], lhsT=lhs,
                            rhs=wvt_sb[k][:, 1024:1280], start=st, stop=sp,
                        )
                    nc.vector.tensor_copy(
                        v_sb[m].rearrange("p (h e) -> p h e", e=P)[:, 0:16, 0:64],
                        main.rearrange("p (h e) -> p h e", e=64),
                    )
                for mm_i in range(2):
                    m = g * 2 + mm_i
                    nc.scalar.copy(
                        v_sb[m].rearrange("p (h e) -> p h e", e=P)[:, 16:20, 0:64],
                        tail[:, mm_i * 256:(mm_i + 1) * 256].rearrange(
                            "p (h e) -> p h e", e=64
                        ),
                    )

            # ---- QK projection unit: j-split halves in a [128,512] held tile
            def make_proj_unit(is_q, i):
                wt_dram = wqt if is_q else wkt
                dst = qt_sb[i] if is_q else kt_sb[i]
                nm = f"w{'q' if is_q else 'k'}{i}"
                st = {}

                def dma_op():
                    wc = w_pool.tile([P, NCT, P], bf16, tag="w", bufs=3, name=nm)
                    nc.sync.dma_start(out=wc, in_=wt_dram[i])
                    st["wc"] = wc

                def mk_mm(j, k):
                    def op():
                        key = f"ps{j}"
                        if key not in st:
                            st[key] = psum.tile(
                                [P, 512], f32, tag="held", bufs=1, name=f"pj{nm}_{j}"
                            )
                        nc.tensor.matmul(
                            st[key], lhsT=st["wc"][:, k, :],
                            rhs=xt_sb[k][:, j * 512:(j + 1) * 512],
                            start=(k == 0), stop=(k == NCT - 1),
                        )
                    return op

                def mk_evict(j):
                    def op():
                        nc.vector.tensor_copy(
                            dst[:, j * 512:(j + 1) * 512], st[f"ps{j}"]
                        )
                    return op

                ops = [dma_op]
                for j in range(2):
                    ops += [mk_mm(j, k) for k in range(NCT)]
                    ops.append(mk_evict(j))
                return ops

            # ---- attention head-pair op-list builder ----
            # Per (qh, kt) stage: two concurrent K=64 scores matmuls (head A
            # rows 0-63 -> cols 0:512, head B rows 64-127 -> cols 512:1024 of
            # one [128,1024] ps tile), one exp N=1024, then lagged PV pairs.
            def make_pair(pair):
                ct = pair
                hA, hB = 2 * pair, 2 * pair + 1
                st = {"sc": {}, "pt": {}, "po": {}}

                def mk_scores(qh, kt):
                    def op():
                        t = psum.tile(
                            [P, S], f32, tag="ps", bufs=2, name=f"sc{pair}_{qh}_{kt}"
                        )
                        st["sc"][(qh, kt)] = t
                        qsl = slice(qh * 512, (qh + 1) * 512)
                        nc.tensor.matmul(
                            t[:, 0:512],
                            lhsT=kt_sb[ct][0:D, kt * P:(kt + 1) * P],
                            rhs=qt_sb[ct][0:D, qsl],
                            start=True, stop=True,
                        )
                        nc.tensor.matmul(
                            t[:, 512:1024],
                            lhsT=kt_sb[ct][D:P, kt * P:(kt + 1) * P],
                            rhs=qt_sb[ct][D:P, qsl],
                            start=True, stop=True,
                        )
                    return op

                def mk_exp(qh, kt):
                    def op():
                        pt = pt_pool.tile(
                            [P, S], bf16, tag="pt", bufs=6, name=f"pt{pair}_{qh}_{kt}"
                        )
                        st["pt"][(qh, kt)] = pt
                        nc.scalar.activation(
                            out=pt, in_=st["sc"][(qh, kt)], func=EXP, scale=SCALE
                        )
                        st["sc"][(qh, kt)] = None
                    return op

                def mk_pv(qh, kt):
                    def op():
                        for hx, h in enumerate((hA, hB)):
                            key = (hx, qh)
                            if key not in st["po"]:
                                st["po"][key] = psum.tile(
                                    [P, 512], f32, tag="pvo", bufs=3,
                                    name=f"po{pair}_{hx}_{qh}",
                                )
                            nc.tensor.matmul(
                                st["po"][key],
                                lhsT=v_sb[kt][:, P * h:P * h + P],
                                rhs=st["pt"][(qh, kt)][
                                    :, hx * 512:(hx + 1) * 512
                                ],
                                start=(kt == 0), stop=(kt == NST - 1),
                            )
                    return op

                def mk_finish(qh):
                    def op():
                        for hx in range(2):
                            po = 64 * hx
                            po_s = po_pool.tile(
                                [P, 512], f32, tag="po", bufs=4,
                                name=f"pos{pair}_{hx}_{qh}",
                            )
                            nc.vector.tensor_copy(po_s, st["po"][(hx, qh)])
                            st["po"][(hx, qh)] = None
                            den_r = n_pool.tile(
                                [D, 512], f32, tag="dr", bufs=4,
                                name=f"dr{pair}_{hx}_{qh}",
                            )
                            nc.vector.tensor_copy(den_r, po_s[64:128, :])
                            nc.vector.reciprocal_approx_fast(out=den_r, in_=den_r)
                            nc.vector.tensor_mul(
                                at_sb[ct][po:po + D, qh * 512:(qh + 1) * 512],
                                po_s[0:64, :], den_r,
                            )
                    return op

                seq = []
                for qh in range(2):
                    for kt in range(NST):
                        seq.append(mk_scores(qh, kt))
                        seq.append(mk_exp(qh, kt))
                        if kt >= DEPTH:
                            seq.append(mk_pv(qh, kt - DEPTH))
                    for kt in range(NST - DEPTH, NST):
                        seq.append(mk_pv(qh, kt))
                    seq.append(mk_finish(qh))
                return seq

            # ---- output projection unit ----
            def make_out_unit(i, prefill=False):
                st = {}

                def dma_op():
                    wc = w_pool.tile([P, NCT, P], bf16, tag="w", bufs=3, name=f"wo{i}")
                    nc.sync.dma_start(out=wc, in_=wot[i])
                    st["wc"] = wc

                def mk_mm_full(k):
                    def op():
                        if "ps" not in st:
                            st["ps"] = psum.tile(
                                [P, S], f32, tag="ps", bufs=2, name=f"py{i}"
                            )
                        lhs = st["wc"][:, k, :]
                        s0, s1 = (k == 0), (k == NCT - 1)
                        nc.tensor.matmul(
                            st["ps"][:, 0:512], lhsT=lhs, rhs=at_sb[k][:, 0:512],
                            start=s0, stop=s1,
                        )
                        nc.tensor.matmul(
                            st["ps"][:, 512:1024], lhsT=lhs,
                            rhs=at_sb[k][:, 512:1024], start=s0, stop=s1,
                        )
                    return op

                def mk_mm_half(j, k):
                    def op():
                        key = f"ps{j}"
                        if key not in st:
                            st[key] = psum.tile(
                                [P, 512], f32, tag="held", bufs=1, name=f"py{i}_{j}"
                            )
                        nc.tensor.matmul(
                            st[key], lhsT=st["wc"][:, k, :],
                            rhs=at_sb[k][:, j * 512:(j + 1) * 512],
                            start=(k == 0), stop=(k == NCT - 1),
                        )
                    return op

                def mk_evict(j, src_key):
                    def op():
                        nch = 2 if i == NCT - 1 else 1
                        w = 512 // nch
                        src = st[src_key]
                        for c_i in range(nch):
                            lo = j * 512 + c_i * w
                            off = lo if src_key == "ps" else c_i * w
                            ys = y_pool.tile(
                                [P, w], f32, tag="y", bufs=3, name=f"ys{i}_{j}_{c_i}"
                            )
                            nc.scalar.activation(
                                out=ys, in_=src[:, off:off + w],
                                func=IDN, bias=bo_sb[:, i:i + 1], scale=1.0,
                            )
                            nc.sync.dma_start(out=yt[i][:, lo:lo + w], in_=ys)
                    return op

                if prefill:
                    # j-split on the held tile so k=0..8 can run during the
                    # last attention pair (at_sb[9] not yet final).
                    pre = [dma_op] + [mk_mm_half(0, k) for k in range(NCT - 1)]
                    post = [mk_mm_half(0, NCT - 1), mk_evict(0, "ps0")]
                    post += [mk_mm_half(1, k) for k in range(NCT)]
                    post.append(mk_evict(1, "ps1"))
                    return pre, post
                ops = [dma_op] + [mk_mm_full(k) for k in range(NCT)]
                ops.append(mk_evict(0, "ps"))
                ops.append(mk_evict(1, "ps"))
                return ops

            # ---- emission ----
            for op in make_proj_unit(True, 0) + make_proj_unit(False, 0):
                op()

            u0_pre, u0_post = make_out_unit(0, prefill=True)

            for pair in range(NCT):
                attn_seq = make_pair(pair)
                if pair < NCT - 1:
                    proj_seq = make_proj_unit(True, pair + 1) + make_proj_unit(
                        False, pair + 1
                    )
                else:
                    proj_seq = u0_pre
                na, npr = len(attn_seq), len(proj_seq)
                j = 0
                for idx, op in enumerate(attn_seq):
                    op()
                    while j * na < (idx + 1) * npr:
                        proj_seq[j]()
                        j += 1
                while j < npr:
                    proj_seq[j]()
                    j += 1

            for op in u0_post:
                op()
            for i in range(1, NCT):
                for op in make_out_unit(i):
                    op()

    nc.finalize()
    return nc


def kernel(**inputs):
    global last_exec_time_ns
    _install_profile_hook()
    import ml_dtypes
    from concourse.bass_utils import run_bass_kernel_spmd

    bfd = ml_dtypes.bfloat16
    hs = np.asarray(inputs["hidden_states"], dtype=np.float32)
    wq = np.asarray(inputs["Wq"], np.float32)
    wk = np.asarray(inputs["Wk"], np.float32)
    wv = np.asarray(inputs["Wv"], np.float32)
    wo = np.asarray(inputs["Wo"], np.float32)
    bo = np.asarray(inputs["bo"], np.float32).reshape(NCT, P).T.copy()

    def pack_io(w):  # [o,c] -> [i, p, k, ocol] for stationary chunks
        return np.ascontiguousarray(
            w.T.reshape(NCT, P, NCT, P).transpose(2, 1, 0, 3).astype(bfd)
        )

    wvt_h = np.ascontiguousarray(wv.T.reshape(NCT, P, C).astype(bfd))
    wqt_h = pack_io(wq)
    wkt_h = pack_io(wk)
    wot_h = pack_io(wo)

    if "nc" not in _cache:
        _cache["nc"] = _build()
    nc = _cache["nc"]

    in_maps = [
        {
            "xt": np.ascontiguousarray(hs[b].T.reshape(NCT, P, S).astype(bfd)),
            "wvt": wvt_h, "wqt": wqt_h, "wkt": wkt_h, "wot": wot_h, "bo": bo,
        }
        for b in range(B)
    ]
    res = run_bass_kernel_spmd(nc, in_maps, list(range(B)))
    last_exec_time_ns = res.exec_time_ns
    out = np.stack(
        [res.results[b]["yt"].reshape(C, S).T for b in range(B)], axis=0
    )
    return np.ascontiguousarray(out.astype(np.float32))
